# revision 14
# baseline (speedup 1.0000x reference)
"""AFM (attentional factorization machine) layer on 8 Trainium2 NeuronCores.

Math: the reference's softmax is over an axis of size 1, so the attention
weights are identically 1 and w1/b1/w2/b2 never affect the output:

    out[b] = sigmoid( (sum_{i<j} e_i * e_j) @ wout + bout )
           = sigmoid( 0.5*((sum_f e_f)^2 - sum_f e_f^2) @ wout + bout )

with e_f = tables[f, sparse[b, f]].  Only the embedding gather is
memory-significant.

Distribution: data-parallel over batch (2048 -> 8 x 256); the 26 embedding
tables are replicated per core (bf16, 83MB).

Per-core kernel:
  - tables uploaded as [325000, 128] bf16: each row is a 256B block holding
    8 consecutive vocab rows (16 bf16 each) of one field.
  - 13 gpsimd dma_gather ops (transpose=True), each covering 2 fields
    (2*12500 = 25000 blocks < int16 range), 512 lookups -> SBUF tile
    [128, 512] where partition p = (subrow r = p>>4, dim k = p&15),
    column = f_local*256 + b_local.
  - host-precomputed one-hot mask over subrows zeroes the 7 unwanted rows
    (VectorE multiply), a second multiply squares the kept values.
  - TensorE matmuls with a constant selector [128,16] (p%16==k) contract the
    partitions, PSUM-accumulating s = sum_f e and sq = sum_f e^2 per (k, b).
  - finale: pooled = s*s - sq (the 0.5 is folded into wout), one matmul with
    0.5*wout, ScalarE sigmoid (+bout), DMA out [1, 256] f32.
"""

import os
import sys

for _p in ("/opt/trn_rl_repo",):
    if _p not in sys.path:
        sys.path.append(_p)

import ml_dtypes
import numpy as np

N_CORES = 8
B = 2048
B_LOC = B // N_CORES  # 256
N_DENSE = 13
N_FIELDS = 26
EMBED = 16
VOCAB = 100000
ROWS_PER_BLK = 8  # 8 rows x 16 bf16 = 256B
BLKS_PER_FIELD = VOCAB // ROWS_PER_BLK  # 12500
FIELDS_PER_GATHER = 2
N_GATHERS = N_FIELDS // FIELDS_PER_GATHER  # 13
NIDX = FIELDS_PER_GATHER * B_LOC  # 512 lookups per gather
TOTAL_SLOTS = N_GATHERS * NIDX  # 6656
ELEM = ROWS_PER_BLK * EMBED  # 128 bf16 per block
TAB_ROWS = N_FIELDS * BLKS_PER_FIELD  # 325000

_COMPILED = None


def _build():
    import concourse.bass as bass  # noqa: F401
    import concourse.tile as tile
    from concourse import bacc, mybir
    from contextlib import ExitStack

    bf16 = mybir.dt.bfloat16
    f32 = mybir.dt.float32
    i16 = mybir.dt.int16

    n_queues = int(os.environ.get("AFM_QUEUES", "4"))
    scratch = int(os.environ.get("AFM_SCRATCH", "16384"))
    nc = bacc.Bacc("TRN2", target_bir_lowering=False, debug=False,
                   num_devices=N_CORES, num_swdge_queues=n_queues,
                   dynamic_dma_scratch_size=scratch)
    t_tab = nc.dram_tensor("tab", [TAB_ROWS, ELEM], bf16, kind="ExternalInput")
    t_idx = nc.dram_tensor("idx", [128, N_GATHERS * (NIDX // 16)], i16,
                           kind="ExternalInput")
    t_mask = nc.dram_tensor("mask", [128, TOTAL_SLOTS], bf16,
                            kind="ExternalInput")
    t_sel = nc.dram_tensor("sel", [128, EMBED], bf16, kind="ExternalInput")
    t_w = nc.dram_tensor("w", [EMBED, 1], bf16, kind="ExternalInput")
    t_bout = nc.dram_tensor("bout", [1, 1], f32, kind="ExternalInput")
    t_out = nc.dram_tensor("out", [1, B_LOC], f32, kind="ExternalOutput")

    with tile.TileContext(nc) as tc, ExitStack() as ctx:
        const_pool = ctx.enter_context(tc.tile_pool(name="const", bufs=1))
        gather_pool = ctx.enter_context(tc.tile_pool(name="gath", bufs=1))
        work_pool = ctx.enter_context(tc.tile_pool(name="work", bufs=1))
        psum_pool = ctx.enter_context(tc.tile_pool(name="ps", bufs=1,
                                                   space="PSUM"))

        idx_sb = const_pool.tile([128, N_GATHERS * (NIDX // 16)], i16)
        nc.sync.dma_start(idx_sb[:], t_idx[:])
        mask_sb = const_pool.tile([128, TOTAL_SLOTS], bf16)
        nc.sync.dma_start(mask_sb[:], t_mask[:])
        sel_sb = const_pool.tile([128, EMBED], bf16)
        nc.sync.dma_start(sel_sb[:], t_sel[:])
        w_sb = const_pool.tile([EMBED, 1], bf16)
        nc.sync.dma_start(w_sb[:], t_w[:])
        bout_sb = const_pool.tile([1, 1], f32)
        nc.sync.dma_start(bout_sb[:], t_bout[:])

        s_psum = psum_pool.tile([EMBED, NIDX], f32, tag="s")
        sq_psum = psum_pool.tile([EMBED, NIDX], f32, tag="sq")

        # PE warm-up + ACT sigmoid table preload while gathers run
        n_warm = int(os.environ.get("AFM_WARMUP", "20"))
        if n_warm:
            warm_psum = psum_pool.tile([EMBED, EMBED], f32, tag="warm")
            for _ in range(n_warm):
                nc.tensor.matmul(warm_psum[:], sel_sb[:], sel_sb[:],
                                 start=True, stop=True)
            warm_act = work_pool.tile([1, 1], f32, tag="warmact")
            nc.scalar.activation(warm_act[:], bout_sb[:],
                                 mybir.ActivationFunctionType.Sigmoid,
                                 bias=0.0, scale=1.0)

        nidx_reg = nc.gpsimd.snap(NIDX)
        for g in range(N_GATHERS):
            c = gather_pool.tile([128, 1, NIDX], bf16, tag=f"c{g}")
            nc.gpsimd.dma_gather(
                c[:],
                t_tab[g * 2 * BLKS_PER_FIELD:(g + 1) * 2 * BLKS_PER_FIELD, :],
                idx_sb[:, g * (NIDX // 16):(g + 1) * (NIDX // 16)],
                NIDX, nidx_reg, ELEM,
                transpose=True,
                single_packet=True,
                queue_num=g % n_queues,
            )
            cm = c[:, 0, :]
            m = work_pool.tile([128, NIDX], bf16, tag=f"m{g}")
            nc.vector.tensor_mul(m[:], cm, mask_sb[:, g * NIDX:(g + 1) * NIDX])
            q = work_pool.tile([128, NIDX], bf16, tag=f"q{g}")
            nc.vector.tensor_mul(q[:], m[:], cm)
            first = g == 0
            last = g == N_GATHERS - 1
            nc.tensor.matmul(s_psum[:], sel_sb[:], m[:],
                             start=first, stop=last)
            nc.tensor.matmul(sq_psum[:], sel_sb[:], q[:],
                             start=first, stop=last)

        s2_sb = work_pool.tile([EMBED, NIDX], f32)
        nc.scalar.copy(s2_sb[:], s_psum[:])
        sq2_sb = work_pool.tile([EMBED, NIDX], f32)
        nc.scalar.copy(sq2_sb[:], sq_psum[:])
        s_sb = work_pool.tile([EMBED, B_LOC], f32)
        nc.vector.tensor_add(s_sb[:], s2_sb[:, 0:B_LOC], s2_sb[:, B_LOC:NIDX])
        sq_sb = work_pool.tile([EMBED, B_LOC], f32)
        nc.vector.tensor_add(sq_sb[:], sq2_sb[:, 0:B_LOC],
                             sq2_sb[:, B_LOC:NIDX])
        ss = work_pool.tile([EMBED, B_LOC], f32)
        nc.vector.tensor_mul(ss[:], s_sb[:], s_sb[:])
        pooled = work_pool.tile([EMBED, B_LOC], bf16)
        nc.vector.tensor_sub(pooled[:], ss[:], sq_sb[:])

        o_psum = psum_pool.tile([1, B_LOC], f32, tag="o")
        nc.tensor.matmul(o_psum[:], w_sb[:], pooled[:],
                         start=True, stop=True)
        out_sb = work_pool.tile([1, B_LOC], f32)
        nc.scalar.activation(out_sb[:], o_psum[:],
                             mybir.ActivationFunctionType.Sigmoid,
                             bias=bout_sb[:], scale=1.0)
        nc.sync.dma_start(t_out[:], out_sb[:])

    nc.compile()
    return nc


def _get_compiled():
    global _COMPILED
    if _COMPILED is None:
        _COMPILED = _build()
    return _COMPILED


def _prep_inputs(inputs, tables, wout, bout):
    """Host-side shard + repack. Returns in_maps for the 8 cores."""
    x = np.asarray(inputs)
    tables = np.asarray(tables, dtype=np.float32)
    wout = np.asarray(wout, dtype=np.float32)
    bout = np.asarray(bout, dtype=np.float32)

    tab_bf16 = np.ascontiguousarray(
        tables.astype(ml_dtypes.bfloat16).reshape(TAB_ROWS, ELEM))
    sel16 = (np.arange(128)[:, None] % EMBED
             == np.arange(EMBED)[None, :]).astype(ml_dtypes.bfloat16)
    w16 = (0.5 * wout).reshape(EMBED, 1).astype(ml_dtypes.bfloat16)
    bout11 = bout.reshape(1, 1).astype(np.float32)
    p_sub = (np.arange(128)[:, None] >> 4)  # subrow held by partition p

    in_maps = []
    for c in range(N_CORES):
        sp = x[c * B_LOC:(c + 1) * B_LOC, N_DENSE:].astype(np.int64)  # [256,26]
        blk = sp >> 3  # block within field
        r = sp & 7     # subrow within block
        idx_cols = []
        rs_all = []
        for g in range(N_GATHERS):
            lb = np.empty((NIDX,), np.int16)
            rr = np.empty((NIDX,), np.int16)
            for fl in range(FIELDS_PER_GATHER):
                f = FIELDS_PER_GATHER * g + fl
                lb[fl * B_LOC:(fl + 1) * B_LOC] = fl * BLKS_PER_FIELD + blk[:, f]
                rr[fl * B_LOC:(fl + 1) * B_LOC] = r[:, f]
            # idx i lives at partition i%16 (replicated x8), free pos i//16
            idx_cols.append(np.tile(lb.reshape(NIDX // 16, 16).T, (8, 1)))
            rs_all.append(rr)
        idx_arr = np.ascontiguousarray(np.concatenate(idx_cols, axis=1))
        rs = np.concatenate(rs_all)  # [6656]
        mask = (p_sub == rs[None, :]).astype(ml_dtypes.bfloat16)
        in_maps.append({"tab": tab_bf16, "idx": idx_arr, "mask": mask,
                        "sel": sel16, "w": w16, "bout": bout11})
    return in_maps


def _ensure_trace_hook():
    """The axon trace path needs antenv.axon_hooks, which this image lacks.
    Register the ctypes-based hook so tracing works instead of crashing."""
    import types

    if "antenv.axon_hooks" in sys.modules:
        return
    try:
        from antenv import axon_hooks  # noqa: F401
        return
    except ImportError:
        pass
    try:
        from trn_agent_boot.trn_boot import _ntff_profile_via_ctypes
        from concourse import bass_utils

        mod = types.ModuleType("antenv.axon_hooks")
        hook = _ntff_profile_via_ctypes("/opt/axon/libaxon_pjrt.so")
        mod.get_axon_ntff_profile_hook = lambda: hook
        mod.set_axon_ntff_profile_hook = lambda h: None
        sys.modules["antenv.axon_hooks"] = mod
        bass_utils.upload_artifacts = lambda tmpdir: tmpdir
    except Exception:
        pass


def _run(inputs, tables, wout, bout, trace=False, tmpdir=None):
    from concourse.bass_utils import run_bass_kernel_spmd

    if trace or os.environ.get("BASS_TRACE"):
        _ensure_trace_hook()
    in_maps = _prep_inputs(inputs, tables, wout, bout)
    nc = _get_compiled()
    res = run_bass_kernel_spmd(nc, in_maps, core_ids=list(range(N_CORES)),
                               trace=trace, tmpdir=tmpdir)
    out = np.concatenate(
        [np.asarray(res.results[c]["out"]).reshape(B_LOC, 1)
         for c in range(N_CORES)], axis=0).astype(np.float32)
    return out, res


def kernel(**inputs) -> np.ndarray:
    out, _ = _run(inputs["inputs"], inputs["tables"], inputs["wout"],
                  inputs["bout"])
    return out
